# revision 29
# baseline (speedup 1.0000x reference)
"""Trainium2 Bass kernel for BConvAttention2d.

v2: all matmul work on PE in fp8 with DoubleRow pair-matmuls (all values
are +-1/0, exactly representable in fp8e4; accumulation is f32 PSUM).

Per core: 4 images as 2 pairs (2 images x 64ch -> 128 partitions).
Per (pair, patch-row group g of 8 patches):
  1. DMA 16 image rows f32; ACT sign -> fp8 binp [128, 8p, 18, 20]
     (patch-local zero borders; interior at rows 1:17, cols 2:18 ).
  2. Depthwise 3x3 per (channel, patch): weights live in the PE stationary
     operand as diagonal matrices (per patch, per tap).  DoubleRow fp8
     pair-matmuls process 2 taps per pass: 4 pairs + 1 single per patch,
     N=256.  No DVE multiplies, no tmp tiles at all.
  3. ACT Sign(psum) -> fp8 bsa (image-padded).  HW Sign(0)=0 matches jnp.
  4. Final dense 3x3 conv 64->64ch: block-diagonal fp8 weights pack both
     images; DoubleRow pairs 2 taps per matmul via an overlapping-window
     pair AP on bsa: 4 pair-MMs + 1 single per 4-row chunk (N=512).
  5. DVE evicts PSUM->SBUF f32; DMA out.

Filters are tiny: sign + diag/block-diag layout happens on host.
"""

import numpy as np
import ml_dtypes

import concourse.bass as bass
import concourse.mybir as mybir
from concourse.tile import TileContext
from concourse.ap import AP
from concourse.bass_utils import run_bass_kernel_spmd

# ---- problem constants (hardcoded per contract) ----
B, C, H, W = 32, 64, 128, 128
N_CORES = 8
B_CORE = B // N_CORES          # 4 images per core
N_PAIRS = B_CORE // 2
PATCH = 16
NP_SIDE = H // PATCH           # 8x8 patch grid
NPATCH = NP_SIDE * NP_SIDE
TAPS = 9
FP8 = mybir.dt.float8e4
F32 = mybir.dt.float32
DR = mybir.MatmulPerfMode.DoubleRow

# depthwise tap pairing: 4 DoubleRow pairs + 1 single (taps as (u, v))
DW_PAIRS = [((0, 0), (0, 2)), ((1, 0), (1, 2)), ((2, 0), (2, 2)),
            ((0, 1), (2, 1))]
DW_SINGLE = (1, 1)
# final-conv tap pairing: anchor tap + element delta to its partner in the
# padded bsa row-major layout (stride 130)
FC_PAIRS = [((0, 0), 1), ((0, 2), 128), ((1, 1), 1), ((2, 0), 1)]
FC_SINGLE = (2, 2)

_CACHED_NC = None


def _pair_ap(a, delta):
    """Insert a [delta, 2] DoubleRow pair dim after the partition dim."""
    dims = [list(p) for p in a.ap]
    return AP(tensor=a.tensor, offset=a.offset,
              ap=[dims[0], [delta, 2]] + dims[1:])


def _split_multiwaits(nc):
    """walrus codegen in this toolchain accepts only ONE embedded sync wait
    per instruction; hoist extras onto preceding NOPs on the same engine."""
    for f in nc.m.functions:
        for blk in f.blocks:
            new_insts = []
            for inst in blk.instructions:
                si = inst.sync_info
                if si is not None and len(si.on_wait) > 1:
                    waits = list(si.on_wait)
                    for w in waits[:-1]:
                        nop = mybir.InstNoOp(
                            name=nc.get_next_instruction_name(), ins=[], outs=[]
                        )
                        nop.engine = inst.engine
                        nop.sync_info = mybir.SyncInfo(on_wait=[w], on_update=[])
                        new_insts.append(nop)
                    inst.sync_info = mybir.SyncInfo(
                        on_wait=[waits[-1]], on_update=list(si.on_update)
                    )
                new_insts.append(inst)
            blk.instructions[:] = new_insts


def _build_nc():
    nc = bass.Bass()
    x = nc.declare_dram_parameter("x", [B_CORE, C, H, W], F32, isOutput=False)
    # diag depthwise weights: [k, group, patch, slot, m]; slots 0..7 are the
    # 4 DoubleRow pairs, slot 8 the single tap
    wd = nc.declare_dram_parameter(
        "wd", [128, NP_SIDE, NP_SIDE, TAPS, 128], FP8, isOutput=False
    )
    # final conv block-diag weights: slots 0..7 pairs, slot 8 single
    wfc = nc.declare_dram_parameter("wfc", [128, TAPS, 128], FP8, isOutput=False)
    y = nc.declare_dram_parameter("y", [B_CORE, C, H, W], F32, isOutput=True)

    with TileContext(nc) as tc:
        with (
            tc.tile_pool(name="persist", bufs=1) as persist,
            tc.tile_pool(name="inp", bufs=4) as inpool,
            tc.tile_pool(name="outp", bufs=6) as outpool,
            tc.tile_pool(name="dwps", bufs=2, space="PSUM") as dwpsum,
            tc.tile_pool(name="cvps", bufs=4, space="PSUM") as cvpsum,
        ):
            wfc_sb = persist.tile([128, TAPS, 128], FP8)
            wd_sb = persist.tile([128, NP_SIDE, NP_SIDE, TAPS, 128], FP8)
            bsa0 = persist.tile([128, H + 2, W + 2], FP8)
            bsa1 = persist.tile([128, H + 2, W + 2], FP8)

            # wfc is small; the 9.2MB wd loads are interleaved into pair 0's
            # group loop (prefetch distance 1) so the first input tile and
            # first weight group land quickly instead of queueing behind the
            # full weight array
            nc.sync.dma_start(out=wfc_sb, in_=wfc[:])

            # binp: patch-padded sign(input); borders must stay zero forever
            binp_tiles = [
                persist.tile([128, NP_SIDE, 18, 20], FP8, name=f"binp{i}")
                for i in range(4)
            ]
            for t in binp_tiles:
                nc.gpsimd.memset(t[:, :, 0, :], 0.0)
                nc.gpsimd.memset(t[:, :, 17, :], 0.0)
                nc.gpsimd.memset(t[:, :, 1:17, 1], 0.0)
                nc.gpsimd.memset(t[:, :, 1:17, 18], 0.0)
            for bs in (bsa0, bsa1):
                nc.gpsimd.memset(bs[:, 0, :], 0.0)
                nc.gpsimd.memset(bs[:, 129, :], 0.0)
                nc.gpsimd.memset(bs[:, 1:129, 0], 0.0)
                nc.gpsimd.memset(bs[:, 1:129, 129], 0.0)

            def conv_chunk(y_pair, bsa, g, cc, evict_eng=None):
                # conv output rows 16g+4cc .. 16g+4cc+3
                y0 = 16 * g + 4 * cc
                pt = cvpsum.tile([128, 4, W], F32, name="cvp")
                for k, ((u, v), d) in enumerate(FC_PAIRS):
                    rhs = _pair_ap(bsa[:, y0 + u:y0 + u + 4, v:v + W], d)
                    nc.tensor.matmul(
                        pt, lhsT=wfc_sb[:, 2 * k:2 * k + 2, :], rhs=rhs,
                        start=(k == 0), stop=False, perf_mode=DR,
                    )
                u, v = FC_SINGLE
                nc.tensor.matmul(
                    pt, lhsT=wfc_sb[:, 8, :],
                    rhs=bsa[:, y0 + u:y0 + u + 4, v:v + W],
                    start=False, stop=True,
                )
                ot = outpool.tile([128, 4, W], F32, name="ot")
                if evict_eng == "scalar":
                    nc.scalar.copy(out=ot, in_=pt)
                else:
                    nc.vector.tensor_copy(out=ot, in_=pt)
                nc.sync.dma_start(
                    out=y_pair[:, :, y0:y0 + 4, :].rearrange(
                        "b c h w -> (b c) h w"
                    ),
                    in_=ot,
                )

            def load_ch(x_pair, g):
                ch = inpool.tile([128, PATCH * W], F32, name="ch")
                nc.sync.dma_start(
                    out=ch,
                    in_=x_pair[:, :, PATCH * g:PATCH * (g + 1), :].rearrange(
                        "b c h w -> (b c) (h w)"
                    ),
                )
                return ch

            # PE warmup: keep the tensor engine busy through the input-DMA
            # wait so the HAM clock gate is released (2.4GHz) when the real
            # matmul stream starts
            warm_ps = dwpsum.tile([128, 4, PATCH, PATCH], F32, name="pg")
            for _ in range(96):
                nc.tensor.matmul(warm_ps[:, 0, 0:4, :], lhsT=wfc_sb[:, 8, :],
                                 rhs=wfc_sb[:, 0, 0:64], start=True, stop=True)

            bi = 0
            seq = [(pi, g) for pi in range(N_PAIRS) for g in range(NP_SIDE)]
            ch_tiles = {0: load_ch(x[0:2], 0)}
            # first weight group split so patch 0's weights land early
            nc.sync.dma_start(out=wd_sb[:, 0, 0:2], in_=wd[:, 0, 0:2])
            ch_tiles[1] = load_ch(x[0:2], 1)
            nc.sync.dma_start(out=wd_sb[:, 0, 2:8], in_=wd[:, 0, 2:8])
            for si, (pi, g) in enumerate(seq):
                x_pair = x[2 * pi:2 * pi + 2]
                y_pair = y[2 * pi:2 * pi + 2]
                bsa = bsa0 if pi % 2 == 0 else bsa1
                ch = ch_tiles.pop(si)
                if si + 2 < len(seq):
                    npi, ng = seq[si + 2]
                    ch_tiles[si + 2] = load_ch(x[2 * npi:2 * npi + 2], ng)
                if pi == 0 and g < NP_SIDE - 1:
                    nc.sync.dma_start(out=wd_sb[:, g + 1], in_=wd[:, g + 1])
                binp = binp_tiles[bi % 4]
                bi += 1
                chv = ch.rearrange("n (i pc j) -> n pc i j", pc=NP_SIDE, j=PATCH)
                if g == 0:
                    # split so hb0's depthwise can start after half the sign
                    nc.scalar.sign(out=binp[:, 0:4, 1:17, 2:18], in_=chv[:, 0:4])
                    nc.scalar.sign(out=binp[:, 4:8, 1:17, 2:18], in_=chv[:, 4:8])
                else:
                    nc.scalar.sign(out=binp[:, :, 1:17, 2:18], in_=chv)

                # ---- final conv, issued BEFORE this group's depthwise so
                # its bsa deps (tracked whole-tile) are already satisfied:
                # chunks 0-2 lag 2 groups, chunk 3 lags 3 (it reads one
                # image row into the following group) ----
                if g >= 3:
                    conv_chunk(y_pair, bsa, g - 3, 3)
                if g >= 2:
                    for cc in range(3):
                        conv_chunk(y_pair, bsa, g - 2, cc)

                # ---- depthwise: diag-weight DoubleRow matmuls ----
                for sb in range(2):
                    pg = dwpsum.tile([128, 4, PATCH, PATCH], F32, name="pg")
                    for pj in range(4):
                        p = 4 * sb + pj
                        for k, (ta, tb) in enumerate(DW_PAIRS):
                            ua, va = ta
                            dlt = (tb[0] - ua) * 20 + (tb[1] - va)
                            rhs = _pair_ap(
                                binp[:, p, ua:ua + 16, 1 + va:17 + va], dlt
                            )
                            nc.tensor.matmul(
                                pg[:, pj],
                                lhsT=wd_sb[:, g, p, 2 * k:2 * k + 2, :],
                                rhs=rhs,
                                start=(k == 0), stop=False, perf_mode=DR,
                            )
                        u, v = DW_SINGLE
                        nc.tensor.matmul(
                            pg[:, pj], lhsT=wd_sb[:, g, p, 8, :],
                            rhs=binp[:, p, u:u + 16, 1 + v:17 + v],
                            start=False, stop=True,
                        )
                    # ---- sign(self_attention) -> image-padded fp8 bsa ----
                    nc.scalar.sign(
                        out=bsa[
                            :,
                            1 + PATCH * g:1 + PATCH * (g + 1),
                            1 + 64 * sb:65 + 64 * sb,
                        ].rearrange("n i (pc j) -> n pc i j", pc=4),
                        in_=pg,
                    )

                if g == NP_SIDE - 1:
                    # remaining conv chunks for this pair's last groups; the
                    # 9-chunk burst outpaces one engine's evictions, so
                    # alternate DVE/ACT here (ACT's sign work is done)
                    tail = [(5, 3), (6, 0), (6, 1), (6, 2), (6, 3),
                            (7, 0), (7, 1), (7, 2), (7, 3)]
                    for i, (tg, cc) in enumerate(tail):
                        conv_chunk(y_pair, bsa, tg, cc,
                                   evict_eng="scalar" if i % 2 else None)

    _split_multiwaits(nc)
    return nc


def _host_weights(patch_filters, output_filters):
    to8 = lambda a: np.ascontiguousarray(a).astype(ml_dtypes.float8_e4m3fn)
    ar = np.arange(128)
    # depthwise diag tiles: wd[c+64s, g, p, slot, m] = w[c, 8g+p, tap] iff m==c+64s
    pfs = np.sign(np.asarray(patch_filters, np.float32))[:, :, 0]  # [c, P, 3, 3]
    pf2 = np.concatenate([pfs, pfs], axis=0)                       # [128, P, 3, 3]
    taps = [t for pr in DW_PAIRS for t in pr] + [DW_SINGLE]        # 9 slot taps
    wd = np.zeros((128, NP_SIDE, NP_SIDE, TAPS, 128), np.float32)
    for s, (u, v) in enumerate(taps):
        wd[ar, :, :, s, ar] = pf2[:, :, u, v].reshape(128, NP_SIDE, NP_SIDE)
    # final conv block-diag: wfc[cin+64s, slot, cout+64s] = ofs[cout, cin, tap]
    ofs = np.sign(np.asarray(output_filters, np.float32))          # [o, i, 3, 3]
    fc_slots = []
    for (ta, d) in FC_PAIRS:
        u, v = ta
        fc_slots.append((u, v))
        off = u * 130 + v + d
        fc_slots.append((off // 130, off % 130))
    fc_slots.append(FC_SINGLE)
    wfc = np.zeros((128, TAPS, 128), np.float32)
    for s, (u, v) in enumerate(fc_slots):
        oft = ofs[:, :, u, v].T                                    # [cin, cout]
        wfc[:C, s, :C] = oft
        wfc[C:, s, C:] = oft
    return to8(wd), to8(wfc)


def kernel(input, k, t, patch_filters, output_filters):
    global _CACHED_NC
    if _CACHED_NC is None:
        _CACHED_NC = _build_nc()
    nc = _CACHED_NC

    x = np.ascontiguousarray(np.asarray(input, np.float32))
    wd, wfc = _host_weights(patch_filters, output_filters)
    in_maps = [
        {"x": np.ascontiguousarray(x[i * B_CORE:(i + 1) * B_CORE]),
         "wd": wd, "wfc": wfc}
        for i in range(N_CORES)
    ]
    res = run_bass_kernel_spmd(nc, in_maps, list(range(N_CORES)))
    return np.concatenate([r["y"] for r in res.results], axis=0)


# revision 34
# speedup vs baseline: 1.1761x; 1.1761x over previous
"""Trainium2 Bass kernel for BConvAttention2d.

v2: all matmul work on PE in fp8 with DoubleRow pair-matmuls (all values
are +-1/0, exactly representable in fp8e4; accumulation is f32 PSUM).

Per core: 4 images as 2 pairs (2 images x 64ch -> 128 partitions).
Per (pair, patch-row group g of 8 patches):
  1. DMA 16 image rows f32; ACT sign -> fp8 binp [128, 8p, 18, 20]
     (patch-local zero borders; interior at rows 1:17, cols 2:18 ).
  2. Depthwise 3x3 per (channel, patch): weights live in the PE stationary
     operand as diagonal matrices (per patch, per tap).  DoubleRow fp8
     pair-matmuls process 2 taps per pass: 4 pairs + 1 single per patch,
     N=256.  No DVE multiplies, no tmp tiles at all.
  3. ACT Sign(psum) -> fp8 bsa (image-padded).  HW Sign(0)=0 matches jnp.
  4. Final dense 3x3 conv 64->64ch: block-diagonal fp8 weights pack both
     images; DoubleRow pairs 2 taps per matmul via an overlapping-window
     pair AP on bsa: 4 pair-MMs + 1 single per 4-row chunk (N=512).
  5. DVE evicts PSUM->SBUF f32; DMA out.

Filters are tiny: sign + diag/block-diag layout happens on host.
"""

import numpy as np
import ml_dtypes

import concourse.bass as bass
import concourse.mybir as mybir
from concourse.tile import TileContext
from concourse.ap import AP
from concourse.bass_utils import run_bass_kernel_spmd

# ---- problem constants (hardcoded per contract) ----
B, C, H, W = 32, 64, 128, 128
N_CORES = 8
B_CORE = B // N_CORES          # 4 images per core
N_PAIRS = B_CORE // 2
PATCH = 16
NP_SIDE = H // PATCH           # 8x8 patch grid
NPATCH = NP_SIDE * NP_SIDE
TAPS = 9
FP8 = mybir.dt.float8e4
F32 = mybir.dt.float32
DR = mybir.MatmulPerfMode.DoubleRow

# depthwise tap pairing: 4 DoubleRow pairs + 1 single (taps as (u, v))
DW_PAIRS = [((0, 0), (0, 2)), ((1, 0), (1, 2)), ((2, 0), (2, 2)),
            ((0, 1), (2, 1))]
DW_SINGLE = (1, 1)
# final-conv tap pairing: anchor tap + element delta to its partner in the
# padded bsa row-major layout (stride 130)
FC_PAIRS = [((0, 0), 1), ((0, 2), 128), ((1, 1), 1), ((2, 0), 1)]
FC_SINGLE = (2, 2)

_CACHED_NC = None


def _pair_ap(a, delta):
    """Insert a [delta, 2] DoubleRow pair dim after the partition dim."""
    dims = [list(p) for p in a.ap]
    return AP(tensor=a.tensor, offset=a.offset,
              ap=[dims[0], [delta, 2]] + dims[1:])


def _split_multiwaits(nc):
    """walrus codegen in this toolchain accepts only ONE embedded sync wait
    per instruction; hoist extras onto preceding NOPs on the same engine."""
    for f in nc.m.functions:
        for blk in f.blocks:
            new_insts = []
            for inst in blk.instructions:
                si = inst.sync_info
                if si is not None and len(si.on_wait) > 1:
                    waits = list(si.on_wait)
                    for w in waits[:-1]:
                        nop = mybir.InstNoOp(
                            name=nc.get_next_instruction_name(), ins=[], outs=[]
                        )
                        nop.engine = inst.engine
                        nop.sync_info = mybir.SyncInfo(on_wait=[w], on_update=[])
                        new_insts.append(nop)
                    inst.sync_info = mybir.SyncInfo(
                        on_wait=[waits[-1]], on_update=list(si.on_update)
                    )
                new_insts.append(inst)
            blk.instructions[:] = new_insts


def _build_nc():
    nc = bass.Bass()
    x = nc.declare_dram_parameter("x", [B_CORE, C, H, W], F32, isOutput=False)
    # diag depthwise weights: [k, group, patch, slot, m]; slots 0..7 are the
    # 4 DoubleRow pairs, slot 8 the single tap
    wd = nc.declare_dram_parameter(
        "wd", [128, NP_SIDE, NP_SIDE, TAPS, 128], FP8, isOutput=False
    )
    # final conv block-diag weights: slots 0..7 pairs, slot 8 single
    wfc = nc.declare_dram_parameter("wfc", [128, TAPS, 128], FP8, isOutput=False)
    y = nc.declare_dram_parameter("y", [B_CORE, C, H, W], F32, isOutput=True)

    with TileContext(nc) as tc:
        with (
            tc.tile_pool(name="persist", bufs=1) as persist,
            tc.tile_pool(name="inp", bufs=4) as inpool,
            tc.tile_pool(name="outp", bufs=6) as outpool,
            tc.tile_pool(name="dwps", bufs=2, space="PSUM") as dwpsum,
            tc.tile_pool(name="cvps", bufs=3, space="PSUM") as cvpsum,
            tc.tile_pool(name="wmps", bufs=1, space="PSUM") as wmpsum,
        ):
            wfc_sb = persist.tile([128, TAPS, 128], FP8)
            wd_sb = persist.tile([128, NP_SIDE, NP_SIDE, TAPS, 128], FP8)
            bsa0 = persist.tile([128, H + 2, W + 2], FP8)
            bsa1 = persist.tile([128, H + 2, W + 2], FP8)

            # wfc is small; the 9.2MB wd loads are interleaved into pair 0's
            # group loop (prefetch distance 1) so the first input tile and
            # first weight group land quickly instead of queueing behind the
            # full weight array
            nc.sync.dma_start(out=wfc_sb, in_=wfc[:])

            # binp: patch-padded sign(input); borders must stay zero forever
            binp_tiles = [
                persist.tile([128, NP_SIDE, 18, 20], FP8, name=f"binp{i}")
                for i in range(4)
            ]
            for t in binp_tiles:
                nc.gpsimd.memset(t[:, :, 0, :], 0.0)
                nc.gpsimd.memset(t[:, :, 17, :], 0.0)
                nc.gpsimd.memset(t[:, :, 1:17, 1], 0.0)
                nc.gpsimd.memset(t[:, :, 1:17, 18], 0.0)
            for bs in (bsa0, bsa1):
                nc.gpsimd.memset(bs[:, 0, :], 0.0)
                nc.gpsimd.memset(bs[:, 129, :], 0.0)
                nc.gpsimd.memset(bs[:, 1:129, 0], 0.0)
                nc.gpsimd.memset(bs[:, 1:129, 129], 0.0)

            def conv_chunk(y_pair, bsa, g, cc, evict_eng=None):
                # conv output rows 16g+4cc .. 16g+4cc+3
                y0 = 16 * g + 4 * cc
                pt = cvpsum.tile([128, 4, W], F32, name="cvp")
                for k, ((u, v), d) in enumerate(FC_PAIRS):
                    rhs = _pair_ap(bsa[:, y0 + u:y0 + u + 4, v:v + W], d)
                    nc.tensor.matmul(
                        pt, lhsT=wfc_sb[:, 2 * k:2 * k + 2, :], rhs=rhs,
                        start=(k == 0), stop=False, perf_mode=DR,
                    )
                u, v = FC_SINGLE
                nc.tensor.matmul(
                    pt, lhsT=wfc_sb[:, 8, :],
                    rhs=bsa[:, y0 + u:y0 + u + 4, v:v + W],
                    start=False, stop=True,
                )
                ot = outpool.tile([128, 4, W], F32, name="ot")
                if evict_eng == "scalar":
                    nc.scalar.copy(out=ot, in_=pt)
                else:
                    nc.vector.tensor_copy(out=ot, in_=pt)
                nc.sync.dma_start(
                    out=y_pair[:, :, y0:y0 + 4, :].rearrange(
                        "b c h w -> (b c) h w"
                    ),
                    in_=ot,
                )

            def load_ch(x_pair, g):
                ch = inpool.tile([128, PATCH * W], F32, name="ch")
                nc.sync.dma_start(
                    out=ch,
                    in_=x_pair[:, :, PATCH * g:PATCH * (g + 1), :].rearrange(
                        "b c h w -> (b c) (h w)"
                    ),
                )
                return ch

            # PE warmup: keep the tensor engine busy through the input-DMA
            # wait so the HAM clock gate is released (2.4GHz) when the real
            # matmul stream starts
            warm_ps = wmpsum.tile([128, 4, W], F32, name="warm")
            for _ in range(96):
                nc.tensor.matmul(warm_ps[:, 0, 0:64], lhsT=wfc_sb[:, 8, :],
                                 rhs=wfc_sb[:, 0, 0:64], start=True, stop=True)

            bi = 0
            seq = [(pi, g) for pi in range(N_PAIRS) for g in range(NP_SIDE)]
            ch_tiles = {0: load_ch(x[0:2], 0)}
            # first weight group split so patch 0's weights land early
            nc.sync.dma_start(out=wd_sb[:, 0, 0:2], in_=wd[:, 0, 0:2])
            ch_tiles[1] = load_ch(x[0:2], 1)
            nc.sync.dma_start(out=wd_sb[:, 0, 2:8], in_=wd[:, 0, 2:8])
            for si, (pi, g) in enumerate(seq):
                x_pair = x[2 * pi:2 * pi + 2]
                y_pair = y[2 * pi:2 * pi + 2]
                bsa = bsa0 if pi % 2 == 0 else bsa1
                ch = ch_tiles.pop(si)
                if si + 2 < len(seq):
                    npi, ng = seq[si + 2]
                    ch_tiles[si + 2] = load_ch(x[2 * npi:2 * npi + 2], ng)
                if pi == 0 and g < NP_SIDE - 1:
                    nc.sync.dma_start(out=wd_sb[:, g + 1], in_=wd[:, g + 1])
                binp = binp_tiles[bi % 4]
                bi += 1
                chv = ch.rearrange("n (i pc j) -> n pc i j", pc=NP_SIDE, j=PATCH)
                if g == 0:
                    # split so hb0's depthwise can start after half the sign
                    nc.scalar.sign(out=binp[:, 0:4, 1:17, 2:18], in_=chv[:, 0:4])
                    nc.scalar.sign(out=binp[:, 4:8, 1:17, 2:18], in_=chv[:, 4:8])
                else:
                    nc.scalar.sign(out=binp[:, :, 1:17, 2:18], in_=chv)

                # ---- final conv, issued BEFORE this group's depthwise so
                # its bsa deps (tracked whole-tile) are already satisfied:
                # chunks 0-2 lag 2 groups, chunk 3 lags 3 (it reads one
                # image row into the following group) ----
                if g >= 3:
                    conv_chunk(y_pair, bsa, g - 3, 3)
                if g >= 2:
                    for cc in range(3):
                        conv_chunk(y_pair, bsa, g - 2, cc)
                if g == NP_SIDE - 1:
                    # group 6's chunks only need sign2(6), already done
                    conv_chunk(y_pair, bsa, 5, 3)
                    for cc in range(3):
                        conv_chunk(y_pair, bsa, 6, cc)

                # ---- depthwise: diag-weight DoubleRow matmuls ----
                for sb in range(2):
                    pg = dwpsum.tile([128, 4, PATCH, PATCH], F32, name="pg")
                    for pj in range(4):
                        p = 4 * sb + pj
                        for k, (ta, tb) in enumerate(DW_PAIRS):
                            ua, va = ta
                            dlt = (tb[0] - ua) * 20 + (tb[1] - va)
                            rhs = _pair_ap(
                                binp[:, p, ua:ua + 16, 1 + va:17 + va], dlt
                            )
                            nc.tensor.matmul(
                                pg[:, pj],
                                lhsT=wd_sb[:, g, p, 2 * k:2 * k + 2, :],
                                rhs=rhs,
                                start=(k == 0), stop=False, perf_mode=DR,
                            )
                        u, v = DW_SINGLE
                        nc.tensor.matmul(
                            pg[:, pj], lhsT=wd_sb[:, g, p, 8, :],
                            rhs=binp[:, p, u:u + 16, 1 + v:17 + v],
                            start=False, stop=True,
                        )
                    # ---- sign(self_attention) -> image-padded fp8 bsa ----
                    nc.scalar.sign(
                        out=bsa[
                            :,
                            1 + PATCH * g:1 + PATCH * (g + 1),
                            1 + 64 * sb:65 + 64 * sb,
                        ].rearrange("n i (pc j) -> n pc i j", pc=4),
                        in_=pg,
                    )

                if g == NP_SIDE - 1:
                    # chunks needing the last group's sign2; the burst
                    # outpaces one engine's evictions, so alternate DVE/ACT
                    # (ACT's sign work is done by now)
                    tail = [(6, 3), (7, 0), (7, 1), (7, 2), (7, 3)]
                    for i, (tg, cc) in enumerate(tail):
                        conv_chunk(y_pair, bsa, tg, cc,
                                   evict_eng="scalar" if i % 2 else None)

    _split_multiwaits(nc)
    return nc


def _host_weights(patch_filters, output_filters):
    to8 = lambda a: np.ascontiguousarray(a).astype(ml_dtypes.float8_e4m3fn)
    ar = np.arange(128)
    # depthwise diag tiles: wd[c+64s, g, p, slot, m] = w[c, 8g+p, tap] iff m==c+64s
    pfs = np.sign(np.asarray(patch_filters, np.float32))[:, :, 0]  # [c, P, 3, 3]
    pf2 = np.concatenate([pfs, pfs], axis=0)                       # [128, P, 3, 3]
    taps = [t for pr in DW_PAIRS for t in pr] + [DW_SINGLE]        # 9 slot taps
    wd = np.zeros((128, NP_SIDE, NP_SIDE, TAPS, 128), np.float32)
    for s, (u, v) in enumerate(taps):
        wd[ar, :, :, s, ar] = pf2[:, :, u, v].reshape(128, NP_SIDE, NP_SIDE)
    # final conv block-diag: wfc[cin+64s, slot, cout+64s] = ofs[cout, cin, tap]
    ofs = np.sign(np.asarray(output_filters, np.float32))          # [o, i, 3, 3]
    fc_slots = []
    for (ta, d) in FC_PAIRS:
        u, v = ta
        fc_slots.append((u, v))
        off = u * 130 + v + d
        fc_slots.append((off // 130, off % 130))
    fc_slots.append(FC_SINGLE)
    wfc = np.zeros((128, TAPS, 128), np.float32)
    for s, (u, v) in enumerate(fc_slots):
        oft = ofs[:, :, u, v].T                                    # [cin, cout]
        wfc[:C, s, :C] = oft
        wfc[C:, s, C:] = oft
    return to8(wd), to8(wfc)


def kernel(input, k, t, patch_filters, output_filters):
    global _CACHED_NC
    if _CACHED_NC is None:
        _CACHED_NC = _build_nc()
    nc = _CACHED_NC

    x = np.ascontiguousarray(np.asarray(input, np.float32))
    wd, wfc = _host_weights(patch_filters, output_filters)
    in_maps = [
        {"x": np.ascontiguousarray(x[i * B_CORE:(i + 1) * B_CORE]),
         "wd": wd, "wfc": wfc}
        for i in range(N_CORES)
    ]
    res = run_bass_kernel_spmd(nc, in_maps, list(range(N_CORES)))
    return np.concatenate([r["y"] for r in res.results], axis=0)


# revision 38
# speedup vs baseline: 1.2123x; 1.0307x over previous
"""Trainium2 Bass kernel for BConvAttention2d.

v2: all matmul work on PE in fp8 with DoubleRow pair-matmuls (all values
are +-1/0, exactly representable in fp8e4; accumulation is f32 PSUM).

Per core: 4 images as 2 pairs (2 images x 64ch -> 128 partitions).
Per (pair, patch-row group g of 8 patches):
  1. DMA 16 image rows f32; ACT sign -> fp8 binp [128, 8p, 18, 20]
     (patch-local zero borders; interior at rows 1:17, cols 2:18 ).
  2. Depthwise 3x3 per (channel, patch): weights live in the PE stationary
     operand as diagonal matrices (per patch, per tap).  DoubleRow fp8
     pair-matmuls process 2 taps per pass: 4 pairs + 1 single per patch,
     N=256.  No DVE multiplies, no tmp tiles at all.
  3. ACT Sign(psum) -> fp8 bsa (image-padded).  HW Sign(0)=0 matches jnp.
  4. Final dense 3x3 conv 64->64ch: block-diagonal fp8 weights pack both
     images; DoubleRow pairs 2 taps per matmul via an overlapping-window
     pair AP on bsa: 4 pair-MMs + 1 single per 4-row chunk (N=512).
  5. DVE evicts PSUM->SBUF f32; DMA out.

Filters are tiny: sign + diag/block-diag layout happens on host.
"""

import numpy as np
import ml_dtypes

import concourse.bass as bass
import concourse.mybir as mybir
from concourse.tile import TileContext
from concourse.ap import AP
from concourse.bass_utils import run_bass_kernel_spmd

# ---- problem constants (hardcoded per contract) ----
B, C, H, W = 32, 64, 128, 128
N_CORES = 8
B_CORE = B // N_CORES          # 4 images per core
N_PAIRS = B_CORE // 2
PATCH = 16
NP_SIDE = H // PATCH           # 8x8 patch grid
NPATCH = NP_SIDE * NP_SIDE
TAPS = 9
FP8 = mybir.dt.float8e4
F32 = mybir.dt.float32
DR = mybir.MatmulPerfMode.DoubleRow

# depthwise tap pairing: 4 DoubleRow pairs + 1 single (taps as (u, v))
DW_PAIRS = [((0, 0), (0, 2)), ((1, 0), (1, 2)), ((2, 0), (2, 2)),
            ((0, 1), (2, 1))]
DW_SINGLE = (1, 1)
# final-conv tap pairing: anchor tap + element delta to its partner in the
# padded bsa row-major layout (stride 130)
FC_PAIRS = [((0, 0), 1), ((0, 2), 128), ((1, 1), 1), ((2, 0), 1)]
FC_SINGLE = (2, 2)

_CACHED_NC = None


def _pair_ap(a, delta):
    """Insert a [delta, 2] DoubleRow pair dim after the partition dim."""
    dims = [list(p) for p in a.ap]
    return AP(tensor=a.tensor, offset=a.offset,
              ap=[dims[0], [delta, 2]] + dims[1:])


def _split_multiwaits(nc):
    """walrus codegen in this toolchain accepts only ONE embedded sync wait
    per instruction; hoist extras onto preceding NOPs on the same engine."""
    for f in nc.m.functions:
        for blk in f.blocks:
            new_insts = []
            for inst in blk.instructions:
                si = inst.sync_info
                if si is not None and len(si.on_wait) > 1:
                    waits = list(si.on_wait)
                    for w in waits[:-1]:
                        nop = mybir.InstNoOp(
                            name=nc.get_next_instruction_name(), ins=[], outs=[]
                        )
                        nop.engine = inst.engine
                        nop.sync_info = mybir.SyncInfo(on_wait=[w], on_update=[])
                        new_insts.append(nop)
                    inst.sync_info = mybir.SyncInfo(
                        on_wait=[waits[-1]], on_update=list(si.on_update)
                    )
                new_insts.append(inst)
            blk.instructions[:] = new_insts


def _build_nc():
    nc = bass.Bass()
    x = nc.declare_dram_parameter("x", [B_CORE, C, H, W], F32, isOutput=False)
    # diag depthwise weights: [k, group, patch, slot, m]; slots 0..7 are the
    # 4 DoubleRow pairs, slot 8 the single tap
    wd = nc.declare_dram_parameter(
        "wd", [128, NP_SIDE, NP_SIDE, TAPS, 128], FP8, isOutput=False
    )
    # final conv block-diag weights: slots 0..7 pairs, slot 8 single
    wfc = nc.declare_dram_parameter("wfc", [128, TAPS, 128], FP8, isOutput=False)
    y = nc.declare_dram_parameter("y", [B_CORE, C, H, W], F32, isOutput=True)

    with TileContext(nc) as tc:
        with (
            tc.tile_pool(name="persist", bufs=1) as persist,
            tc.tile_pool(name="inp", bufs=4) as inpool,
            tc.tile_pool(name="outp", bufs=6) as outpool,
            tc.tile_pool(name="dwps", bufs=2, space="PSUM") as dwpsum,
            tc.tile_pool(name="cvps", bufs=3, space="PSUM") as cvpsum,
            tc.tile_pool(name="wmps", bufs=1, space="PSUM") as wmpsum,
        ):
            wfc_sb = persist.tile([128, TAPS, 128], FP8)
            wd_sb = persist.tile([128, NP_SIDE, NP_SIDE, TAPS, 128], FP8)
            bsa0 = persist.tile([128, H + 2, W + 2], FP8)
            bsa1 = persist.tile([128, H + 2, W + 2], FP8)

            # wfc is small; the 9.2MB wd loads are interleaved into pair 0's
            # group loop (prefetch distance 1) so the first input tile and
            # first weight group land quickly instead of queueing behind the
            # full weight array
            nc.sync.dma_start(out=wfc_sb, in_=wfc[:])

            # binp: patch-padded sign(input); borders must stay zero forever
            binp_tiles = [
                persist.tile([128, NP_SIDE, 18, 20], FP8, name=f"binp{i}")
                for i in range(4)
            ]
            for t in binp_tiles:
                nc.gpsimd.memset(t[:, :, 0, :], 0.0)
                nc.gpsimd.memset(t[:, :, 17, :], 0.0)
                nc.gpsimd.memset(t[:, :, 1:17, 1], 0.0)
                nc.gpsimd.memset(t[:, :, 1:17, 18], 0.0)
            for bs in (bsa0, bsa1):
                nc.gpsimd.memset(bs[:, 0, :], 0.0)
                nc.gpsimd.memset(bs[:, 129, :], 0.0)
                nc.gpsimd.memset(bs[:, 1:129, 0], 0.0)
                nc.gpsimd.memset(bs[:, 1:129, 129], 0.0)

            cv_cnt = [0]

            def conv_chunk(y_pair, bsa, g, cc):
                # conv output rows 16g+4cc .. 16g+4cc+3
                y0 = 16 * g + 4 * cc
                pt = cvpsum.tile([128, 4, W], F32, name="cvp")
                for k, ((u, v), d) in enumerate(FC_PAIRS):
                    rhs = _pair_ap(bsa[:, y0 + u:y0 + u + 4, v:v + W], d)
                    nc.tensor.matmul(
                        pt, lhsT=wfc_sb[:, 2 * k:2 * k + 2, :], rhs=rhs,
                        start=(k == 0), stop=False, perf_mode=DR,
                    )
                u, v = FC_SINGLE
                nc.tensor.matmul(
                    pt, lhsT=wfc_sb[:, 8, :],
                    rhs=bsa[:, y0 + u:y0 + u + 4, v:v + W],
                    start=False, stop=True,
                )
                ot = outpool.tile([128, 4, W], F32, name="ot")
                # alternate DVE/ACT: one engine alone cannot keep up with the
                # FC chunk burst cadence through the PSUM pool recycle
                if cv_cnt[0] % 2 == 0:
                    nc.vector.tensor_copy(out=ot, in_=pt)
                else:
                    nc.scalar.copy(out=ot, in_=pt)
                cv_cnt[0] += 1
                nc.sync.dma_start(
                    out=y_pair[:, :, y0:y0 + 4, :].rearrange(
                        "b c h w -> (b c) h w"
                    ),
                    in_=ot,
                )

            def load_ch(x_pair, g):
                ch = inpool.tile([128, PATCH * W], F32, name="ch")
                nc.sync.dma_start(
                    out=ch,
                    in_=x_pair[:, :, PATCH * g:PATCH * (g + 1), :].rearrange(
                        "b c h w -> (b c) (h w)"
                    ),
                )
                return ch

            # PE warmup: keep the tensor engine busy through the input-DMA
            # wait so the HAM clock gate is released (2.4GHz) when the real
            # matmul stream starts
            warm_ps = wmpsum.tile([128, 4, W], F32, name="warm")
            for _ in range(96):
                nc.tensor.matmul(warm_ps[:, 0, 0:64], lhsT=wfc_sb[:, 8, :],
                                 rhs=wfc_sb[:, 0, 0:64], start=True, stop=True)

            bi = 0
            seq = [(pi, g) for pi in range(N_PAIRS) for g in range(NP_SIDE)]
            ch_tiles = {0: load_ch(x[0:2], 0)}
            # first weight group split so patch 0's weights land early
            nc.sync.dma_start(out=wd_sb[:, 0, 0:2], in_=wd[:, 0, 0:2])
            nc.sync.dma_start(out=wd_sb[:, 0, 2:8], in_=wd[:, 0, 2:8])
            ch_tiles[1] = load_ch(x[0:2], 1)
            for si, (pi, g) in enumerate(seq):
                x_pair = x[2 * pi:2 * pi + 2]
                y_pair = y[2 * pi:2 * pi + 2]
                bsa = bsa0 if pi % 2 == 0 else bsa1
                ch = ch_tiles.pop(si)
                if si + 2 < len(seq):
                    npi, ng = seq[si + 2]
                    ch_tiles[si + 2] = load_ch(x[2 * npi:2 * npi + 2], ng)
                if pi == 0 and g < NP_SIDE - 1:
                    nc.sync.dma_start(out=wd_sb[:, g + 1], in_=wd[:, g + 1])
                binp = binp_tiles[bi % 4]
                bi += 1
                chv = ch.rearrange("n (i pc j) -> n pc i j", pc=NP_SIDE, j=PATCH)
                if g == 0:
                    # split so hb0's depthwise can start after half the sign
                    nc.scalar.sign(out=binp[:, 0:4, 1:17, 2:18], in_=chv[:, 0:4])
                    nc.scalar.sign(out=binp[:, 4:8, 1:17, 2:18], in_=chv[:, 4:8])
                else:
                    nc.scalar.sign(out=binp[:, :, 1:17, 2:18], in_=chv)

                # ---- final conv, issued BEFORE this group's depthwise so
                # its bsa deps (tracked whole-tile) are already satisfied:
                # chunks 0-2 lag 2 groups, chunk 3 lags 3 (it reads one
                # image row into the following group) ----
                if g >= 3:
                    conv_chunk(y_pair, bsa, g - 3, 3)
                if g >= 2:
                    for cc in range(3):
                        conv_chunk(y_pair, bsa, g - 2, cc)
                if g == NP_SIDE - 1:
                    # group 6's chunks only need sign2(6), already done
                    conv_chunk(y_pair, bsa, 5, 3)
                    for cc in range(3):
                        conv_chunk(y_pair, bsa, 6, cc)

                # ---- depthwise: diag-weight DoubleRow matmuls ----
                for sb in range(2):
                    pg = dwpsum.tile([128, 4, PATCH, PATCH], F32, name="pg")
                    for pj in range(4):
                        p = 4 * sb + pj
                        for k, (ta, tb) in enumerate(DW_PAIRS):
                            ua, va = ta
                            dlt = (tb[0] - ua) * 20 + (tb[1] - va)
                            rhs = _pair_ap(
                                binp[:, p, ua:ua + 16, 1 + va:17 + va], dlt
                            )
                            nc.tensor.matmul(
                                pg[:, pj],
                                lhsT=wd_sb[:, g, p, 2 * k:2 * k + 2, :],
                                rhs=rhs,
                                start=(k == 0), stop=False, perf_mode=DR,
                            )
                        u, v = DW_SINGLE
                        nc.tensor.matmul(
                            pg[:, pj], lhsT=wd_sb[:, g, p, 8, :],
                            rhs=binp[:, p, u:u + 16, 1 + v:17 + v],
                            start=False, stop=True,
                        )
                    # ---- sign(self_attention) -> image-padded fp8 bsa ----
                    nc.scalar.sign(
                        out=bsa[
                            :,
                            1 + PATCH * g:1 + PATCH * (g + 1),
                            1 + 64 * sb:65 + 64 * sb,
                        ].rearrange("n i (pc j) -> n pc i j", pc=4),
                        in_=pg,
                    )

                if g == NP_SIDE - 1:
                    # chunks needing the last group's sign2; the burst
                    # outpaces one engine's evictions, so alternate DVE/ACT
                    # (ACT's sign work is done by now)
                    for tg, cc in [(6, 3), (7, 0), (7, 1), (7, 2), (7, 3)]:
                        conv_chunk(y_pair, bsa, tg, cc)

    _split_multiwaits(nc)
    return nc


def _host_weights(patch_filters, output_filters):
    to8 = lambda a: np.ascontiguousarray(a).astype(ml_dtypes.float8_e4m3fn)
    ar = np.arange(128)
    # depthwise diag tiles: wd[c+64s, g, p, slot, m] = w[c, 8g+p, tap] iff m==c+64s
    pfs = np.sign(np.asarray(patch_filters, np.float32))[:, :, 0]  # [c, P, 3, 3]
    pf2 = np.concatenate([pfs, pfs], axis=0)                       # [128, P, 3, 3]
    taps = [t for pr in DW_PAIRS for t in pr] + [DW_SINGLE]        # 9 slot taps
    wd = np.zeros((128, NP_SIDE, NP_SIDE, TAPS, 128), np.float32)
    for s, (u, v) in enumerate(taps):
        wd[ar, :, :, s, ar] = pf2[:, :, u, v].reshape(128, NP_SIDE, NP_SIDE)
    # final conv block-diag: wfc[cin+64s, slot, cout+64s] = ofs[cout, cin, tap]
    ofs = np.sign(np.asarray(output_filters, np.float32))          # [o, i, 3, 3]
    fc_slots = []
    for (ta, d) in FC_PAIRS:
        u, v = ta
        fc_slots.append((u, v))
        off = u * 130 + v + d
        fc_slots.append((off // 130, off % 130))
    fc_slots.append(FC_SINGLE)
    wfc = np.zeros((128, TAPS, 128), np.float32)
    for s, (u, v) in enumerate(fc_slots):
        oft = ofs[:, :, u, v].T                                    # [cin, cout]
        wfc[:C, s, :C] = oft
        wfc[C:, s, C:] = oft
    return to8(wd), to8(wfc)


def kernel(input, k, t, patch_filters, output_filters):
    global _CACHED_NC
    if _CACHED_NC is None:
        _CACHED_NC = _build_nc()
    nc = _CACHED_NC

    x = np.ascontiguousarray(np.asarray(input, np.float32))
    wd, wfc = _host_weights(patch_filters, output_filters)
    in_maps = [
        {"x": np.ascontiguousarray(x[i * B_CORE:(i + 1) * B_CORE]),
         "wd": wd, "wfc": wfc}
        for i in range(N_CORES)
    ]
    res = run_bass_kernel_spmd(nc, in_maps, list(range(N_CORES)))
    return np.concatenate([r["y"] for r in res.results], axis=0)
